# revision 1
# baseline (speedup 1.0000x reference)
"""Trainium2 Bass kernel for nn_CysInteractLayer (GNN message-passing layer).

out = BN(lrelu(lrelu(lrelu([ef | nf[src]+nf[dst]] @ W1 + b1) @ W2 + b2) @ W3 + b3))

Sharding: edges across 8 NeuronCores; node_feats/weights replicated
(as per-half local node tables so dma_gather's int16 indices suffice);
BN batch stats all-reduced across cores on-device.
"""
import numpy as np
import ml_dtypes

import concourse.bass as bass
import concourse.bacc as bacc
import concourse.tile as tile
from concourse import mybir
from concourse.bass_utils import run_bass_kernel_spmd
from concourse.masks import make_identity

F32 = mybir.dt.float32
BF16 = mybir.dt.bfloat16
I16 = mybir.dt.int16

# problem constants (hardcoded per harness contract)
V, E = 50000, 400000
ND, ED, OUT = 128, 64, 128
IN = ND + ED
NEG_SLOPE = 0.01
BN_EPS = 1e-5

NCORES = 8
ES = E // NCORES            # 50000 edges per core


class Cfg:
    """Geometry of the per-core kernel; small instances used for sim tests."""

    def __init__(self, es=ES, halves=2, calls_per_half=7, blocks_per_call=7,
                 tn=32768, e_total=E, use_ttr=False, per_tile_groups=False,
                 skip_collective=False, skip_gather=False, gq=1):
        self.use_ttr = use_ttr
        self.per_tile_groups = per_tile_groups
        self.skip_collective = skip_collective
        self.skip_gather = skip_gather
        self.gq = gq
        self.ES = es                        # valid edges per core
        self.H = halves
        self.C = calls_per_half
        self.B = blocks_per_call
        self.BLK = 512
        self.TN = tn                        # local table rows (padded)
        self.E_TOTAL = e_total
        self.HP = self.C * self.B * self.BLK   # padded edges per half
        self.EP = self.H * self.HP             # padded edges per core
        self.HV = es // halves                 # valid edges per half
        assert es % halves == 0
        assert self.HP >= self.HV
        self.NB = self.H * self.C * self.B     # total blocks
        self.NCALL = self.B * self.BLK         # idxs per gather call
        self.WCOLS = self.NCALL // 16          # wrapped idx cols per call

    def block_valid(self, b):
        """Valid columns in global block b (rest is padding)."""
        h, lb = divmod(b, self.C * self.B)
        lo = lb * self.BLK
        return int(np.clip(self.HV - lo, 0, self.BLK))


CFG = Cfg()

_PROG_CACHE = {}


def build_program(cfg):
    nc = bacc.Bacc(None, target_bir_lowering=False, num_swdge_queues=4)

    tabs = [nc.declare_dram_parameter(f"tab{h}", [cfg.TN, ND], BF16, isOutput=False)
            for h in range(cfg.H)]
    srcw = nc.declare_dram_parameter("srcw", [128, 2 * cfg.H * cfg.C * cfg.WCOLS], I16, isOutput=False)
    dstw = nc.declare_dram_parameter("dstw", [128, 2 * cfg.H * cfg.C * cfg.WCOLS], I16, isOutput=False)
    eft = nc.declare_dram_parameter("eft", [ED, cfg.EP], BF16, isOutput=False)
    w1e = nc.declare_dram_parameter("w1e", [ED, OUT], BF16, isOutput=False)
    w1m = nc.declare_dram_parameter("w1m", [ND, OUT], BF16, isOutput=False)
    w2 = nc.declare_dram_parameter("w2", [OUT, OUT], BF16, isOutput=False)
    w3 = nc.declare_dram_parameter("w3", [OUT, OUT], BF16, isOutput=False)
    bias = nc.declare_dram_parameter("bias", [128, 3], F32, isOutput=False)   # b1|b2|b3
    gb = nc.declare_dram_parameter("gb", [128, 2], F32, isOutput=False)       # gamma|beta
    out = nc.declare_dram_parameter("out", [cfg.EP, OUT], F32, isOutput=True)

    LR = mybir.ActivationFunctionType.Lrelu
    IDENT = mybir.ActivationFunctionType.Identity

    with tile.TileContext(nc) as tc:
        with (
            tc.tile_pool(name="singles", bufs=1) as singles,
            tc.tile_pool(name="hres", bufs=1) as hres,
            tc.tile_pool(name="gat", bufs=2) as gat,
            tc.tile_pool(name="work", bufs=3) as work,
            tc.tile_pool(name="zps", bufs=4, space="PSUM") as zps,
            tc.tile_pool(name="ops", bufs=3, space="PSUM") as ops,
            tc.tile_pool(name="dram", bufs=1, space="DRAM") as dram,
        ):
            # ---- static state ------------------------------------------------
            w1e_t = singles.tile([ED, OUT], BF16)
            nc.sync.dma_start(out=w1e_t[:], in_=w1e[:, :])
            w1m_t = singles.tile([ND, OUT], BF16)
            nc.sync.dma_start(out=w1m_t[:], in_=w1m[:, :])
            w2_t = singles.tile([OUT, OUT], BF16)
            nc.sync.dma_start(out=w2_t[:], in_=w2[:, :])
            w3_t = singles.tile([OUT, OUT], BF16)
            nc.sync.dma_start(out=w3_t[:], in_=w3[:, :])
            bias_t = singles.tile([128, 3], F32)
            nc.sync.dma_start(out=bias_t[:], in_=bias[:, :])
            gb_t = singles.tile([128, 2], F32)
            nc.sync.dma_start(out=gb_t[:], in_=gb[:, :])
            srcw_t = singles.tile([128, 2 * cfg.H * cfg.C * cfg.WCOLS], I16)
            nc.sync.dma_start(out=srcw_t[:], in_=srcw[:, :])
            dstw_t = singles.tile([128, 2 * cfg.H * cfg.C * cfg.WCOLS], I16)
            nc.sync.dma_start(out=dstw_t[:], in_=dstw[:, :])
            ident_f32 = singles.tile([128, 128], F32)
            make_identity(nc, ident_f32[:])

            h3res = hres.tile([128, cfg.EP], BF16)
            sum_stripe = singles.tile([128, cfg.NB], F32)
            sq_stripe = singles.tile([128, cfg.NB], F32)

            # ---- phase 1: gather + MLP + stats -------------------------------
            for h in range(cfg.H):
                for c in range(cfg.C):
                    wofs = (h * cfg.C + c) * cfg.WCOLS
                    gsd = gat.tile([128, 1, 2 * cfg.NCALL], BF16, tag="gsd")
                    if cfg.skip_gather:
                        nc.vector.memset(gsd[:], 0.25)
                    else:
                        nc.gpsimd.dma_gather(
                            out_ap=gsd[:], in_ap=tabs[h][:, :],
                            idxs_ap=srcw_t[:, 2 * wofs:2 * (wofs + cfg.WCOLS)],
                            num_idxs=2 * cfg.NCALL, num_idxs_reg=2 * cfg.NCALL,
                            elem_size=ND, transpose=True, single_packet=False,
                        )
                    gs = gsd[:, :, :cfg.NCALL]
                    gd = gsd[:, :, cfg.NCALL:]
                    call_e0 = (h * cfg.C + c) * cfg.NCALL
                    eft_t = gat.tile([ED, cfg.NCALL], BF16, tag="eft")
                    nc.sync.dma_start(out=eft_t[:], in_=eft[:, call_e0:call_e0 + cfg.NCALL])

                    for b in range(cfg.B):
                        gb_idx = (h * cfg.C + c) * cfg.B + b
                        vb = cfg.block_valid(gb_idx)
                        if vb == 0:
                            continue
                        co = b * cfg.BLK          # col offset within call
                        e0 = call_e0 + co         # global (padded) edge offset
                        zp = zps.tile([128, cfg.BLK], F32, tag="z")
                        nc.tensor.matmul(zp[:, :], lhsT=w1m_t[:], rhs=gs[:, 0, co:co + cfg.BLK],
                                         start=True, stop=False)
                        nc.tensor.matmul(zp[:, :], lhsT=w1m_t[:], rhs=gd[:, 0, co:co + cfg.BLK],
                                         start=False, stop=False)
                        nc.tensor.matmul(zp[:, :], lhsT=w1e_t[:], rhs=eft_t[:, co:co + cfg.BLK],
                                         start=False, stop=True)
                        h1 = work.tile([128, cfg.BLK], BF16, tag="h1")
                        nc.scalar.activation(out=h1[:], in_=zp[:], func=LR,
                                             bias=bias_t[:, 0:1], scale=1.0, alpha=NEG_SLOPE)
                        zp2 = zps.tile([128, cfg.BLK], F32, tag="z")
                        nc.tensor.matmul(zp2[:, :], lhsT=w2_t[:], rhs=h1[:], start=True, stop=True)
                        h2 = work.tile([128, cfg.BLK], BF16, tag="h2")
                        nc.scalar.activation(out=h2[:], in_=zp2[:], func=LR,
                                             bias=bias_t[:, 1:2], scale=1.0, alpha=NEG_SLOPE)
                        zp3 = zps.tile([128, cfg.BLK], F32, tag="z")
                        nc.tensor.matmul(zp3[:, :], lhsT=w3_t[:], rhs=h2[:], start=True, stop=True)
                        h3 = h3res[:, e0:e0 + cfg.BLK]
                        nc.scalar.activation(out=h3[:, :vb], in_=zp3[:, :vb], func=LR,
                                             bias=bias_t[:, 2:3], scale=1.0, alpha=NEG_SLOPE,
                                             accum_out=sum_stripe[:, gb_idx:gb_idx + 1])
                        if vb < cfg.BLK:
                            nc.vector.memset(h3[:, vb:], 0.0)
                        sq_scr = work.tile([128, cfg.BLK], BF16, tag="sq")
                        nc.vector.tensor_mul(out=sq_scr[:], in0=h3[:, :], in1=h3[:, :])
                        nc.vector.tensor_reduce(
                            out=sq_stripe[:, gb_idx:gb_idx + 1], in_=sq_scr[:],
                            axis=mybir.AxisListType.X, op=mybir.AluOpType.add)

            # ---- stats + allreduce ------------------------------------------
            st2 = singles.tile([128, 2], F32)
            nc.vector.tensor_reduce(out=st2[:, 0:1], in_=sum_stripe[:],
                                    axis=mybir.AxisListType.X, op=mybir.AluOpType.add)
            nc.vector.tensor_reduce(out=st2[:, 1:2], in_=sq_stripe[:],
                                    axis=mybir.AxisListType.X, op=mybir.AluOpType.add)
            cc_in = dram.tile([128, 2], F32)
            cc_out = dram.tile([128, 2], F32)
            nc.gpsimd.dma_start(out=cc_in[:], in_=st2[:])
            if cfg.skip_collective:
                nc.gpsimd.dma_start(out=cc_out[:], in_=cc_in[:])
            else:
                nc.gpsimd.collective_compute(
                    "AllReduce", mybir.AluOpType.add,
                    replica_groups=[list(range(NCORES))],
                    ins=[cc_in.opt()], outs=[cc_out.opt()],
                )
            gst = singles.tile([128, 2], F32)
            nc.gpsimd.dma_start(out=gst[:], in_=cc_out[:])

            inv_e = 1.0 / cfg.E_TOTAL
            mean_t = singles.tile([128, 1], F32)
            nc.scalar.mul(out=mean_t[:], in_=gst[:, 0:1], mul=inv_e)
            msq_t = singles.tile([128, 1], F32)
            nc.scalar.mul(out=msq_t[:], in_=gst[:, 1:2], mul=inv_e)
            var_t = singles.tile([128, 1], F32)
            nc.vector.tensor_tensor(out=var_t[:], in0=mean_t[:], in1=mean_t[:],
                                    op=mybir.AluOpType.mult)
            nc.vector.tensor_tensor(out=var_t[:], in0=msq_t[:], in1=var_t[:],
                                    op=mybir.AluOpType.subtract)
            eps_t = singles.tile([128, 1], F32)
            nc.vector.memset(eps_t[:], BN_EPS)
            sd_t = singles.tile([128, 1], F32)
            nc.scalar.activation(out=sd_t[:], in_=var_t[:],
                                 func=mybir.ActivationFunctionType.Sqrt,
                                 bias=eps_t[:], scale=1.0)
            rstd_t = singles.tile([128, 1], F32)
            nc.vector.reciprocal(out=rstd_t[:], in_=sd_t[:])
            s_t = singles.tile([128, 1], F32)
            nc.vector.tensor_tensor(out=s_t[:], in0=rstd_t[:], in1=gb_t[:, 0:1],
                                    op=mybir.AluOpType.mult)
            t_t = singles.tile([128, 1], F32)
            nc.vector.tensor_tensor(out=t_t[:], in0=s_t[:], in1=mean_t[:],
                                    op=mybir.AluOpType.mult)
            nc.vector.tensor_tensor(out=t_t[:], in0=gb_t[:, 1:2], in1=t_t[:],
                                    op=mybir.AluOpType.subtract)

            # ---- phase 2: affine + transpose + store ------------------------
            for gb_idx in range(cfg.NB):
                if cfg.block_valid(gb_idx) == 0:
                    continue
                e0 = gb_idx * cfg.BLK
                u = work.tile([128, cfg.BLK], F32, tag="u")
                nc.scalar.activation(out=u[:], in_=h3res[:, e0:e0 + cfg.BLK],
                                     func=IDENT, bias=t_t[:], scale=s_t[:])
                op = ops.tile([128, cfg.BLK], F32, tag="op")
                nsub = cfg.BLK // 128
                for t in range(nsub):
                    st_, sp_ = ((True, True) if cfg.per_tile_groups
                                else (t == 0, t == nsub - 1))
                    nc.tensor.matmul(op[:, t * 128:(t + 1) * 128],
                                     lhsT=u[:, t * 128:(t + 1) * 128], rhs=ident_f32[:],
                                     is_transpose=True, start=st_, stop=sp_)
                ob = work.tile([128, cfg.BLK], F32, tag="ob")
                nc.vector.tensor_copy(out=ob[:], in_=op[:])
                dst_ap = out[e0:e0 + cfg.BLK, :].rearrange("(t p) f -> p t f", p=128)
                nc.sync.dma_start(out=dst_ap, in_=ob[:].rearrange("p (t f) -> p t f", f=128))
    nc.compile()
    return nc


def get_program(cfg):
    key = (cfg.ES, cfg.H, cfg.C, cfg.B, cfg.TN, cfg.E_TOTAL,
           cfg.use_ttr, cfg.per_tile_groups, cfg.skip_collective, cfg.skip_gather, cfg.gq)
    if key not in _PROG_CACHE:
        _PROG_CACHE[key] = build_program(cfg)
    return _PROG_CACHE[key]


def _wrap_idx(flat, cfg):
    """int16 flat idxs [n] -> wrapped [128, n/16] layout (i at [i%16, i//16], x8)."""
    w = flat.reshape(-1, 16).T.astype(np.int16)      # [16, n/16]
    return np.tile(w, (8, 1))                        # [128, n/16]


def host_prep(node_feats, edge_feats, src, dst, W1, b1, W2, b2, W3, b3, gamma, beta,
              cfg=None):
    cfg = cfg or CFG
    nfb = np.asarray(node_feats, np.float32).astype(ml_dtypes.bfloat16)
    efb = np.asarray(edge_feats, np.float32).astype(ml_dtypes.bfloat16)
    src = np.asarray(src)
    dst = np.asarray(dst)
    W1 = np.asarray(W1, np.float32)

    w1e = W1[:ED].astype(ml_dtypes.bfloat16)
    w1m = W1[ED:].astype(ml_dtypes.bfloat16)
    w2b = np.asarray(W2, np.float32).astype(ml_dtypes.bfloat16)
    w3b = np.asarray(W3, np.float32).astype(ml_dtypes.bfloat16)
    bias = np.stack([np.asarray(b1, np.float32),
                     np.asarray(b2, np.float32),
                     np.asarray(b3, np.float32)], axis=1)          # [128, 3]
    gb = np.stack([np.asarray(gamma, np.float32),
                   np.asarray(beta, np.float32)], axis=1)          # [128, 2]

    in_maps = []
    for c in range(NCORES):
        base = c * cfg.ES
        tabs, sws, dws = [], [], []
        for h in range(cfg.H):
            lo = base + h * cfg.HV
            s_h = src[lo:lo + cfg.HV]
            d_h = dst[lo:lo + cfg.HV]
            u = np.unique(np.concatenate([s_h, d_h]))
            assert len(u) <= cfg.TN, f"local table overflow: {len(u)} > {cfg.TN}"
            assert len(u) <= 32768, "int16 index overflow"
            tab = np.zeros((cfg.TN, ND), ml_dtypes.bfloat16)
            tab[:len(u)] = nfb[u]
            tabs.append(tab)
            s16 = np.searchsorted(u, s_h).astype(np.int16)
            d16 = np.searchsorted(u, d_h).astype(np.int16)
            pad = cfg.HP - cfg.HV
            if pad:
                s16 = np.concatenate([s16, np.zeros(pad, np.int16)])
                d16 = np.concatenate([d16, np.zeros(pad, np.int16)])
            # one wrapped array per gather call: src block then dst block merged
            for cl in range(cfg.C):
                sws.append(_wrap_idx(s16[cl * cfg.NCALL:(cl + 1) * cfg.NCALL], cfg))
                sws.append(_wrap_idx(d16[cl * cfg.NCALL:(cl + 1) * cfg.NCALL], cfg))
                dws.append(np.zeros((128, cfg.WCOLS), np.int16))
                dws.append(np.zeros((128, cfg.WCOLS), np.int16))
        srcw = np.concatenate(sws, axis=1)
        dstw = np.concatenate(dws, axis=1)
        # edge feats, transposed + per-half padding
        eftc = np.zeros((ED, cfg.EP), ml_dtypes.bfloat16)
        for h in range(cfg.H):
            lo = base + h * cfg.HV
            eftc[:, h * cfg.HP:h * cfg.HP + cfg.HV] = efb[lo:lo + cfg.HV].T
        im = {"srcw": srcw, "dstw": dstw, "eft": eftc,
              "w1e": w1e, "w1m": w1m, "w2": w2b, "w3": w3b,
              "bias": bias, "gb": gb}
        for h in range(cfg.H):
            im[f"tab{h}"] = tabs[h]
        in_maps.append(im)
    return in_maps


def assemble_output(results, cfg=None):
    cfg = cfg or CFG
    out = np.empty((NCORES * cfg.ES, OUT), np.float32)
    for c in range(NCORES):
        oc = np.asarray(results[c]["out"])
        for h in range(cfg.H):
            lo = c * cfg.ES + h * cfg.HV
            out[lo:lo + cfg.HV] = oc[h * cfg.HP:h * cfg.HP + cfg.HV]
    return out


def kernel(**inputs):
    cfg = CFG
    nc = get_program(cfg)
    in_maps = host_prep(**inputs, cfg=cfg)
    res = run_bass_kernel_spmd(nc, in_maps, list(range(NCORES)))
    return assemble_output(res.results, cfg)



# revision 7
# speedup vs baseline: 2.5792x; 2.5792x over previous
"""Trainium2 Bass kernel for nn_CysInteractLayer (GNN message-passing layer).

out = BN(lrelu(lrelu(lrelu([ef | nf[src]+nf[dst]] @ W1 + b1) @ W2 + b2) @ W3 + b3))

Sharding: edges across 8 NeuronCores; node_feats/weights replicated.
The node table lives in SBUF feature-major ([128, V]) and per-edge
gathers are SBUF column-gathers via the HW-decoded IndirectCopy pool
instruction (the Q7 software dma_gather was the old bottleneck).
BN batch stats all-reduced across cores on-device; h3 staged in DRAM
between the stats barrier and the affine pass; final output stays
feature-major and is transposed on host.
"""
import numpy as np
import ml_dtypes

import concourse.bass as bass
import concourse.bacc as bacc
import concourse.tile as tile
from concourse import mybir
from concourse.bass_utils import run_bass_kernel_spmd

F32 = mybir.dt.float32
BF16 = mybir.dt.bfloat16
U16 = mybir.dt.uint16

# problem constants (hardcoded per harness contract)
V, E = 50000, 400000
ND, ED, OUT = 128, 64, 128
IN = ND + ED
NEG_SLOPE = 0.01
BN_EPS = 1e-5

NCORES = 8
ES = E // NCORES            # 50000 edges per core


class Cfg:
    """Geometry of the per-core kernel."""

    def __init__(self, es=ES, calls=14, blocks_per_call=7, e_total=E,
                 skip_collective=False):
        self.skip_collective = skip_collective
        self.ES = es                        # valid edges per core
        self.C = calls
        self.B = blocks_per_call
        self.BLK = 512
        self.TN = 50048                     # node table cols (V padded)
        self.E_TOTAL = e_total
        self.NCALL = self.B * self.BLK      # edges per eft/h3 call chunk
        self.EP = self.C * self.NCALL       # padded edges per core
        self.NB = self.C * self.B           # total blocks
        self.WCOLS = 2 * self.BLK // 16     # wrapped idx cols per block (src+dst)
        assert self.EP >= es

    def block_valid(self, b):
        """Valid columns in global block b (rest is padding)."""
        lo = b * self.BLK
        return int(np.clip(self.ES - lo, 0, self.BLK))


CFG = Cfg()

_PROG_CACHE = {}


def build_program(cfg):
    nc = bacc.Bacc(None, target_bir_lowering=False, num_swdge_queues=4)

    tabT = nc.declare_dram_parameter("tabT", [128, cfg.TN], BF16, isOutput=False)
    srcw = nc.declare_dram_parameter("srcw", [128, cfg.NB * cfg.WCOLS], U16, isOutput=False)
    eft = nc.declare_dram_parameter("eft", [ED, cfg.EP], BF16, isOutput=False)
    w1e = nc.declare_dram_parameter("w1e", [ED, OUT], BF16, isOutput=False)
    w1m = nc.declare_dram_parameter("w1m", [ND, OUT], BF16, isOutput=False)
    w2 = nc.declare_dram_parameter("w2", [OUT, OUT], BF16, isOutput=False)
    w3 = nc.declare_dram_parameter("w3", [OUT, OUT], BF16, isOutput=False)
    bias = nc.declare_dram_parameter("bias", [128, 3], F32, isOutput=False)   # b1|b2|b3
    gb = nc.declare_dram_parameter("gb", [128, 2], F32, isOutput=False)       # gamma|beta
    out = nc.declare_dram_parameter("out", [128, cfg.EP], F32, isOutput=True)

    LR = mybir.ActivationFunctionType.Lrelu
    IDENT = mybir.ActivationFunctionType.Identity

    with tile.TileContext(nc) as tc:
        with (
            tc.tile_pool(name="singles", bufs=1) as singles,
            tc.tile_pool(name="tabp", bufs=1) as tabp,
            tc.tile_pool(name="gat", bufs=2) as gat,
            tc.tile_pool(name="work", bufs=3) as work,
            tc.tile_pool(name="ph2", bufs=2) as ph2,
            tc.tile_pool(name="zps", bufs=4, space="PSUM") as zps,
            tc.tile_pool(name="dram", bufs=1, space="DRAM") as dram,
        ):
            # ---- static state ------------------------------------------------
            w1e_t = singles.tile([ED, OUT], BF16)
            nc.sync.dma_start(out=w1e_t[:], in_=w1e[:, :])
            w1m_t = singles.tile([ND, OUT], BF16)
            nc.sync.dma_start(out=w1m_t[:], in_=w1m[:, :])
            w2_t = singles.tile([OUT, OUT], BF16)
            nc.sync.dma_start(out=w2_t[:], in_=w2[:, :])
            w3_t = singles.tile([OUT, OUT], BF16)
            nc.sync.dma_start(out=w3_t[:], in_=w3[:, :])
            bias_t = singles.tile([128, 3], F32)
            nc.sync.dma_start(out=bias_t[:], in_=bias[:, :])
            gb_t = singles.tile([128, 2], F32)
            nc.sync.dma_start(out=gb_t[:], in_=gb[:, :])
            srcw_t = singles.tile([128, cfg.NB * cfg.WCOLS], U16)
            nc.sync.dma_start(out=srcw_t[:], in_=srcw[:, :])
            tab_t = tabp.tile([128, cfg.TN], BF16)
            nc.sync.dma_start(out=tab_t[:], in_=tabT[:, :])

            h3d = dram.tile([128, cfg.EP], BF16)
            sum_stripe = singles.tile([128, cfg.NB], F32)
            sq_stripe = singles.tile([128, cfg.NB], F32)

            # ---- phase 1: gather + MLP + stats -------------------------------
            for c in range(cfg.C):
                call_e0 = c * cfg.NCALL
                eft_t = gat.tile([ED, cfg.NCALL], BF16, tag="eft")
                nc.sync.dma_start(out=eft_t[:], in_=eft[:, call_e0:call_e0 + cfg.NCALL])
                h3c = gat.tile([128, cfg.NCALL], BF16, tag="h3c")

                for b in range(cfg.B):
                    gb_idx = c * cfg.B + b
                    vb = cfg.block_valid(gb_idx)
                    if vb == 0:
                        continue
                    co = b * cfg.BLK          # col offset within call
                    wofs = gb_idx * cfg.WCOLS
                    gsd = gat.tile([128, 2 * cfg.BLK], BF16, tag="gsd")
                    nc.gpsimd.indirect_copy(
                        out=gsd[:], data=tab_t[:],
                        idxs=srcw_t[:, wofs:wofs + cfg.WCOLS],
                        i_know_ap_gather_is_preferred=True,
                    )
                    zp = zps.tile([128, cfg.BLK], F32, tag="z")
                    nc.tensor.matmul(zp[:, :], lhsT=w1m_t[:], rhs=gsd[:, :cfg.BLK],
                                     start=True, stop=False)
                    nc.tensor.matmul(zp[:, :], lhsT=w1m_t[:],
                                     rhs=gsd[:, cfg.BLK:],
                                     start=False, stop=False)
                    nc.tensor.matmul(zp[:, :], lhsT=w1e_t[:], rhs=eft_t[:, co:co + cfg.BLK],
                                     start=False, stop=True)
                    h1 = work.tile([128, cfg.BLK], BF16, tag="h1")
                    nc.scalar.activation(out=h1[:], in_=zp[:], func=LR,
                                         bias=bias_t[:, 0:1], scale=1.0, alpha=NEG_SLOPE)
                    zp2 = zps.tile([128, cfg.BLK], F32, tag="z")
                    nc.tensor.matmul(zp2[:, :], lhsT=w2_t[:], rhs=h1[:], start=True, stop=True)
                    h2 = work.tile([128, cfg.BLK], BF16, tag="h2")
                    nc.scalar.activation(out=h2[:], in_=zp2[:], func=LR,
                                         bias=bias_t[:, 1:2], scale=1.0, alpha=NEG_SLOPE)
                    zp3 = zps.tile([128, cfg.BLK], F32, tag="z")
                    nc.tensor.matmul(zp3[:, :], lhsT=w3_t[:], rhs=h2[:], start=True, stop=True)
                    h3 = h3c[:, co:co + cfg.BLK]
                    nc.scalar.activation(out=h3[:, :vb], in_=zp3[:, :vb], func=LR,
                                         bias=bias_t[:, 2:3], scale=1.0, alpha=NEG_SLOPE,
                                         accum_out=sum_stripe[:, gb_idx:gb_idx + 1])
                    if vb < cfg.BLK:
                        nc.vector.memset(h3[:, vb:], 0.0)
                    sq_scr = work.tile([128, cfg.BLK], BF16, tag="sq")
                    nc.vector.tensor_mul(out=sq_scr[:], in0=h3[:, :], in1=h3[:, :])
                    nc.vector.tensor_reduce(
                        out=sq_stripe[:, gb_idx:gb_idx + 1], in_=sq_scr[:],
                        axis=mybir.AxisListType.X, op=mybir.AluOpType.add)
                nc.sync.dma_start(out=h3d[:, call_e0:call_e0 + cfg.NCALL], in_=h3c[:])

            # ---- stats + allreduce ------------------------------------------
            st2 = singles.tile([128, 2], F32)
            nc.vector.tensor_reduce(out=st2[:, 0:1], in_=sum_stripe[:],
                                    axis=mybir.AxisListType.X, op=mybir.AluOpType.add)
            nc.vector.tensor_reduce(out=st2[:, 1:2], in_=sq_stripe[:],
                                    axis=mybir.AxisListType.X, op=mybir.AluOpType.add)
            cc_in = dram.tile([128, 2], F32)
            cc_out = dram.tile([128, 2], F32)
            nc.gpsimd.dma_start(out=cc_in[:], in_=st2[:])
            if cfg.skip_collective:
                nc.gpsimd.dma_start(out=cc_out[:], in_=cc_in[:])
            else:
                nc.gpsimd.collective_compute(
                    "AllReduce", mybir.AluOpType.add,
                    replica_groups=[list(range(NCORES))],
                    ins=[cc_in.opt()], outs=[cc_out.opt()],
                )
            gst = singles.tile([128, 2], F32)
            nc.gpsimd.dma_start(out=gst[:], in_=cc_out[:])

            inv_e = 1.0 / cfg.E_TOTAL
            mean_t = singles.tile([128, 1], F32)
            nc.scalar.mul(out=mean_t[:], in_=gst[:, 0:1], mul=inv_e)
            msq_t = singles.tile([128, 1], F32)
            nc.scalar.mul(out=msq_t[:], in_=gst[:, 1:2], mul=inv_e)
            var_t = singles.tile([128, 1], F32)
            nc.vector.tensor_tensor(out=var_t[:], in0=mean_t[:], in1=mean_t[:],
                                    op=mybir.AluOpType.mult)
            nc.vector.tensor_tensor(out=var_t[:], in0=msq_t[:], in1=var_t[:],
                                    op=mybir.AluOpType.subtract)
            eps_t = singles.tile([128, 1], F32)
            nc.vector.memset(eps_t[:], BN_EPS)
            sd_t = singles.tile([128, 1], F32)
            nc.scalar.activation(out=sd_t[:], in_=var_t[:],
                                 func=mybir.ActivationFunctionType.Sqrt,
                                 bias=eps_t[:], scale=1.0)
            rstd_t = singles.tile([128, 1], F32)
            nc.vector.reciprocal(out=rstd_t[:], in_=sd_t[:])
            s_t = singles.tile([128, 1], F32)
            nc.vector.tensor_tensor(out=s_t[:], in0=rstd_t[:], in1=gb_t[:, 0:1],
                                    op=mybir.AluOpType.mult)
            t_t = singles.tile([128, 1], F32)
            nc.vector.tensor_tensor(out=t_t[:], in0=s_t[:], in1=mean_t[:],
                                    op=mybir.AluOpType.mult)
            nc.vector.tensor_tensor(out=t_t[:], in0=gb_t[:, 1:2], in1=t_t[:],
                                    op=mybir.AluOpType.subtract)

            # ---- phase 2: affine + store (feature-major; host transposes) ---
            CH = cfg.NCALL // 2               # phase-2 chunk cols
            for k in range(2 * cfg.C):
                e0 = k * CH
                h3s = ph2.tile([128, CH], BF16, tag="h3s")
                nc.sync.dma_start(out=h3s[:], in_=h3d[:, e0:e0 + CH])
                u = ph2.tile([128, CH], F32, tag="u")
                if k % 2 == 0:
                    nc.scalar.activation(out=u[:], in_=h3s[:], func=IDENT,
                                         bias=t_t[:], scale=s_t[:])
                else:
                    nc.vector.tensor_scalar(out=u[:], in0=h3s[:],
                                            scalar1=s_t[:], scalar2=t_t[:],
                                            op0=mybir.AluOpType.mult,
                                            op1=mybir.AluOpType.add)
                nc.sync.dma_start(out=out[:, e0:e0 + CH], in_=u[:])
    nc.compile()
    return nc


def get_program(cfg):
    key = (cfg.ES, cfg.C, cfg.B, cfg.TN, cfg.E_TOTAL, cfg.skip_collective)
    if key not in _PROG_CACHE:
        _PROG_CACHE[key] = build_program(cfg)
    return _PROG_CACHE[key]


def _wrap_idx(flat):
    """uint16 flat idxs [n] -> wrapped [128, n/16] layout (i at [i%16, i//16], x8)."""
    w = flat.reshape(-1, 16).T.astype(np.uint16)     # [16, n/16]
    return np.tile(w, (8, 1))                        # [128, n/16]


def host_prep(node_feats, edge_feats, src, dst, W1, b1, W2, b2, W3, b3, gamma, beta,
              cfg=None):
    cfg = cfg or CFG
    nfb = np.asarray(node_feats, np.float32).astype(ml_dtypes.bfloat16)
    efb = np.asarray(edge_feats, np.float32).astype(ml_dtypes.bfloat16)
    src = np.asarray(src)
    dst = np.asarray(dst)
    W1 = np.asarray(W1, np.float32)

    tabT = np.zeros((128, cfg.TN), ml_dtypes.bfloat16)
    tabT[:, :V] = nfb.T

    w1e = W1[:ED].astype(ml_dtypes.bfloat16)
    w1m = W1[ED:].astype(ml_dtypes.bfloat16)
    w2b = np.asarray(W2, np.float32).astype(ml_dtypes.bfloat16)
    w3b = np.asarray(W3, np.float32).astype(ml_dtypes.bfloat16)
    bias = np.stack([np.asarray(b1, np.float32),
                     np.asarray(b2, np.float32),
                     np.asarray(b3, np.float32)], axis=1)          # [128, 3]
    gbv = np.stack([np.asarray(gamma, np.float32),
                    np.asarray(beta, np.float32)], axis=1)         # [128, 2]

    in_maps = []
    for c in range(NCORES):
        base = c * cfg.ES
        s16 = src[base:base + cfg.ES].astype(np.uint16)
        d16 = dst[base:base + cfg.ES].astype(np.uint16)
        pad = cfg.EP - cfg.ES
        if pad:
            s16 = np.concatenate([s16, np.zeros(pad, np.uint16)])
            d16 = np.concatenate([d16, np.zeros(pad, np.uint16)])
        sws = []
        for bi in range(cfg.NB):
            lo = bi * cfg.BLK
            blk = np.concatenate([s16[lo:lo + cfg.BLK], d16[lo:lo + cfg.BLK]])
            sws.append(_wrap_idx(blk))
        srcw = np.concatenate(sws, axis=1)
        eftc = np.zeros((ED, cfg.EP), ml_dtypes.bfloat16)
        eftc[:, :cfg.ES] = efb[base:base + cfg.ES].T
        im = {"tabT": tabT, "srcw": srcw, "eft": eftc,
              "w1e": w1e, "w1m": w1m, "w2": w2b, "w3": w3b,
              "bias": bias, "gb": gbv}
        in_maps.append(im)
    return in_maps


def assemble_output(results, cfg=None):
    cfg = cfg or CFG
    out = np.empty((NCORES * cfg.ES, OUT), np.float32)
    for c in range(NCORES):
        oc = np.asarray(results[c]["out"])            # [128, EP] f32
        out[c * cfg.ES:(c + 1) * cfg.ES] = oc[:, :cfg.ES].T
    return out


def kernel(**inputs):
    cfg = CFG
    nc = get_program(cfg)
    in_maps = host_prep(**inputs, cfg=cfg)
    res = run_bass_kernel_spmd(nc, in_maps, list(range(NCORES)))
    return assemble_output(res.results, cfg)
